# revision 17
# baseline (speedup 1.0000x reference)
"""Trainium2 Bass kernel for the Gaussian density calculator.

density[g] = sum_a mask_a * sum_n aw[e_a,n] * exp(bw[e_a,n] * ||g - X_a||^2)

Strategy (self-contained; hardcoded for 8 NeuronCores):
 - Host: drop masked atoms (they contribute exactly 0), spatially sort the
   grid points into 128-point tiles, and for every tile build the list of
   (atom, gaussian) pairs whose contribution can exceed exp(-CUT) anywhere
   in the tile (|bw| * d_min^2 <= CUT, d_min = distance from atom to the
   tile's bounding box).  Dropped terms are < 1e-6 relative -- far below
   fp32 resolution of the result.
 - The exponent is affine in per-point features:
       arg = bw*|g'|^2 - 2bw*(g'.X') + bw*|X'|^2 + log(aw)
           = [ |g'|^2, g'x, g'y, g'z, 1 ] . W[:, pair]
   (coordinates recentred per tile, aw folded into the exponent as log(aw)).
   On device: K=5 matmul per tile -> exp on ScalarE -> pair-sum on VectorE.
 - fp32-accurate matmul on the bf16 PE datapath: both operands split into
   3 bf16 components; the 6 cross products with |error| >= 2^-27 stack
   along the contraction dim (K = 30 <= 32, one PE row group).
 - Tiles are dealt to the 8 cores by workload rank (SPMD: identical
   instruction stream, near-balanced data); similar-sized tiles batch into
   one PSUM bank so a single ACTIVATE + one 3D-AP TENSOR_REDUCE serve the
   whole batch.  All matmuls of a batch share one PE row group (HW
   requirement for bank sharing); groups rotate across batches.
 - Operands stream in column-chunks so compute overlaps the input DMA.
"""
import numpy as np
import ml_dtypes

import concourse.bacc as bacc
import concourse.tile as tile
from concourse import mybir
from concourse.bass_utils import run_bass_kernel_spmd

P = 128
NCORES = 8
EXCLUDED_ELEM = 5
CUT = 16.0
MM_MAX = 512            # cols per matmul (one PSUM bank, fp32)
ITEM_MAX = 1024         # pair cols per batch item (2 banks, singleton batch)
BATCH_MAX = 512         # pair cols per multi-item batch (one PSUM bank --
                        # a matmul output must never straddle a bank)
BATCH_SLOTS = 16        # max items per batch
NCHUNKS = 2             # input DMA column chunks (compute/DMA overlap)
NEG_BIG = -1e30
NGROUPS = 3             # usable PE row groups for K<=32: {0,32,64}
BF16 = ml_dtypes.bfloat16


def _split3(x):
    a0 = x.astype(BF16)
    r1 = x - a0.astype(np.float64)
    a1 = r1.astype(BF16)
    r2 = r1 - a1.astype(np.float64)
    a2 = r2.astype(BF16)
    return a0, a1, a2


def _g_band(g5, k15):
    g0, g1, g2 = _split3(g5)
    if k15:                       # g exactly bf16: products (00),(01),(02)
        return np.concatenate([g0, g0, g0], axis=0)
    return np.concatenate([g0, g1, g2, g0, g0, g1], axis=0)


def _w_band(w5, k15):
    w0, w1, w2 = _split3(w5)
    if k15:
        return np.concatenate([w0, w1, w2], axis=0)
    return np.concatenate([w0, w1, w0, w1, w2, w0], axis=0)


def _prepare(grid_points, X, aw_table, bw_table, elements, C_expand):
    gp = grid_points.astype(np.float64)
    Ng = gp.shape[0]

    mask = (elements != EXCLUDED_ELEM) & (C_expand == 1)
    Xa = X.astype(np.float64)[mask]
    el = elements[mask]
    aw = aw_table.astype(np.float64)[el]
    bw = bw_table.astype(np.float64)[el]
    with np.errstate(divide="ignore", invalid="ignore"):
        logaw = np.where(aw > 0, np.log(np.maximum(aw, 1e-300)), NEG_BIG)

    # ---- spatial sort into tiles of 128 points ----
    ntiles = -(-Ng // P)
    ntiles = -(-ntiles // NCORES) * NCORES
    cell = np.floor(gp / np.array([2.0, 2.0, 4.0]))
    order = np.lexsort((cell[:, 2], cell[:, 1], cell[:, 0]))
    npad = ntiles * P - Ng
    order_padded = np.concatenate([order, np.full(npad, order[-1], np.int64)])
    gp_s = gp[order_padded].reshape(ntiles, P, 3)

    lo = gp_s.min(axis=1)
    hi = gp_s.max(axis=1)
    center = (lo + hi) / 2

    # ---- per-tile (atom, gaussian) pair selection ----
    d = np.maximum(lo[:, None, :] - Xa[None], Xa[None] - hi[:, None, :])
    d2 = (np.maximum(d, 0.0) ** 2).sum(-1)
    incl = (-bw)[None] * d2[:, :, None] <= CUT            # [T,Na,6]
    cnt = incl.reshape(ntiles, -1).sum(1)

    # ---- deal tiles to cores by workload rank ----
    nslots = ntiles // NCORES
    rank = np.argsort(-cnt, kind="stable")
    tilemap = rank.reshape(nslots, NCORES)                # [k, c] -> tile id
    pad_k = np.maximum(cnt[tilemap].max(1), 2)
    pad_k = ((pad_k + 1) // 2) * 2

    # ---- split slots into items (<= ITEM_MAX pair cols each) ----
    items = []                                            # [slot, q0, size]
    for k in range(nslots):
        rem, q0 = int(pad_k[k]), 0
        while rem > 0:
            s = min(rem, ITEM_MAX)
            items.append([k, q0, s])
            q0 += s
            rem -= s

    # ---- pack items into batches (greedy; items arrive size-sorted) ----
    batches = []                                          # dict(n, items)
    cur, cur_n = [], 0
    for it in items:
        n = max(cur_n, it[2])
        if cur and (len(cur) >= BATCH_SLOTS or (len(cur) + 1) * n > BATCH_MAX):
            batches.append(dict(n=cur_n, items=cur))
            cur, cur_n = [], 0
            n = it[2]
        cur.append(it)
        cur_n = n
    if cur:
        batches.append(dict(n=cur_n, items=cur))

    # ---- assign chunks, acc columns, G/W cols; emit matmul descriptors ----
    # chunk 0 is small so compute starts early; the rest stream behind it
    total_g = sum(-(-b["n"] // MM_MAX) * len(b["items"]) for b in batches)
    fracs = [0.34, 0.28, 0.22, 0.16]
    bounds = np.cumsum([f * total_g for f in fracs])

    chunks = []                                           # per chunk: counters
    acccol = 0
    gdone = 0
    slot_cols = [[] for _ in range(nslots)]
    for bidx, b in enumerate(batches):
        n = b["n"]
        grp = bidx % NGROUPS
        nm = -(-n // MM_MAX)
        ci = min(int(np.searchsorted(bounds, gdone, side="right")),
                 len(fracs) - 1)
        gdone += nm * len(b["items"])
        while len(chunks) <= ci:
            chunks.append(dict(g=[0] * NGROUPS, w=[0] * NGROUPS))
        ch = chunks[ci]
        b["chunk"] = ci
        for item in b["items"]:
            k, q0, size = item
            item_mms = []
            c0 = 0
            while c0 < n:
                sz = min(MM_MAX, n - c0)
                item_mms.append(dict(grp=grp, gcol=ch["g"][grp],
                                     woff=ch["w"][grp], sz=sz, c0=c0))
                ch["g"][grp] += 1
                ch["w"][grp] += sz
                c0 += sz
            item.append(item_mms)
            item.append(acccol)
            slot_cols[k].append(acccol)
            acccol += 1
    ncols = acccol
    gw_i = [max(c["g"]) * P for c in chunks]              # per-chunk G width
    ww_i = [max(c["w"]) for c in chunks]                  # per-chunk W width
    nchunks = len(chunks)

    # ---- per-core operand arrays ----
    g5_all = np.empty((ntiles, 5, P))
    gprime = gp_s - center[:, None, :]
    g5_all[:, 0] = (gprime ** 2).sum(-1)
    g5_all[:, 1:4] = np.swapaxes(gprime, 1, 2)
    g5_all[:, 4] = 1.0
    # grid features exactly bf16-representable (regular lattice) -> the G
    # split degenerates and K drops from 30 to 15 (halves operand DMA)
    k15 = bool(np.all(g5_all == g5_all.astype(BF16).astype(np.float64)))
    krows = 15 if k15 else 30

    pair_an = [np.nonzero(incl[t]) for t in range(ntiles)]
    Gc = [[np.zeros((3 * krows, gw_i[i]), BF16) for i in range(nchunks)]
          for _ in range(NCORES)]
    Wc = [[np.zeros((3 * krows, ww_i[i]), BF16) for i in range(nchunks)]
          for _ in range(NCORES)]
    gband_cache = {}
    for b in batches:
        n, ci = b["n"], b["chunk"]
        for k, q0, size, item_mms, _col in b["items"]:
            for c in range(NCORES):
                t = int(tilemap[k, c])
                if t not in gband_cache:
                    gband_cache[t] = _g_band(g5_all[t], k15)
                aa, nn = pair_an[t]
                a_it = aa[q0:q0 + n]
                n_it = nn[q0:q0 + n]
                mi = a_it.shape[0]
                w5 = np.empty((5, n))
                w5[:, mi:] = np.array([0, 0, 0, 0, NEG_BIG])[:, None]
                if mi:
                    Xp = Xa[a_it] - center[t]
                    bwi = bw[a_it, n_it]
                    w5[0, :mi] = bwi
                    w5[1:4, :mi] = -2.0 * bwi * Xp.T
                    w5[4, :mi] = bwi * (Xp ** 2).sum(-1) + logaw[a_it, n_it]
                wb = _w_band(w5, k15)
                for mm in item_mms:
                    p0 = krows * mm["grp"]
                    Gc[c][ci][p0:p0 + krows,
                              mm["gcol"] * P:(mm["gcol"] + 1) * P] = \
                        gband_cache[t]
                    Wc[c][ci][p0:p0 + krows,
                              mm["woff"]:mm["woff"] + mm["sz"]] = \
                        wb[:, mm["c0"]:mm["c0"] + mm["sz"]]

    meta = dict(
        nslots=nslots, ncols=ncols, batches=batches, slot_cols=slot_cols,
        gw_i=gw_i, ww_i=ww_i, nchunks=nchunks, pad_k=pad_k, krows=krows,
        tilemap=tilemap, order_padded=order_padded, Ng=Ng, ntiles=ntiles,
    )
    return Gc, Wc, meta


def _build_program(meta):
    nc = bacc.Bacc("TRN2", target_bir_lowering=False, debug=False,
                   num_devices=NCORES)
    ncols = meta["ncols"]
    gw_i, ww_i = meta["gw_i"], meta["ww_i"]
    nchunks = meta["nchunks"]
    krows = meta["krows"]
    g_d = [nc.dram_tensor(f"g{i}", [3 * krows, gw_i[i]], mybir.dt.bfloat16,
                          kind="ExternalInput") for i in range(nchunks)]
    w_d = [nc.dram_tensor(f"w{i}", [3 * krows, ww_i[i]], mybir.dt.bfloat16,
                          kind="ExternalInput") for i in range(nchunks)]
    out_d = nc.dram_tensor("out", [P, ncols], mybir.dt.float32,
                           kind="ExternalOutput")

    with tile.TileContext(nc) as tc:
        with (
            tc.tile_pool(name="data", bufs=1) as data,
            tc.tile_pool(name="ps", bufs=4, space="PSUM") as ps,
            tc.tile_pool(name="work", bufs=4) as work,
        ):
            g_sb, w_sb = [], []
            for i in range(nchunks):
                gt = data.tile([P, gw_i[i]], mybir.dt.bfloat16, tag=f"g{i}")
                wt = data.tile([P, ww_i[i]], mybir.dt.bfloat16, tag=f"w{i}")
                # move only the 3 used 32-aligned bands, spread over the two
                # fast HWDGE queues (each serves issue order -> chunk 0
                # lands first); gpsimd's slow SWDGE only carries the output
                for g in range(NGROUPS):
                    qg = (nc.sync, nc.scalar)[g % 2]
                    qw = (nc.scalar, nc.sync)[g % 2]
                    qg.dma_start(gt[32 * g:32 * g + krows, :],
                                 g_d[i][krows * g:krows * (g + 1), :])
                    qw.dma_start(wt[32 * g:32 * g + krows, :],
                                 w_d[i][krows * g:krows * (g + 1), :])
                g_sb.append(gt)
                w_sb.append(wt)
            acc = data.tile([P, ncols], mybir.dt.float32)
            for b in meta["batches"]:
                n, bi, ci = b["n"], b["items"], b["chunk"]
                B = len(bi)
                ps3 = ps.tile([P, B, n], mybir.dt.float32, tag="arg")
                e3 = work.tile([P, B, n], mybir.dt.float16, tag="e")
                for bidx, (k, q0, size, item_mms, _col) in enumerate(bi):
                    for mm in item_mms:
                        p0 = 32 * mm["grp"]
                        nc.tensor.matmul(
                            ps3[:, bidx, mm["c0"]:mm["c0"] + mm["sz"]],
                            g_sb[ci][p0:p0 + krows,
                                     mm["gcol"] * P:(mm["gcol"] + 1) * P],
                            w_sb[ci][p0:p0 + krows,
                                     mm["woff"]:mm["woff"] + mm["sz"]],
                            start=True, stop=True,
                        )
                nc.scalar.activation(out=e3[:], in_=ps3[:],
                                     func=mybir.ActivationFunctionType.Exp)
                col0 = bi[0][4]
                nc.vector.tensor_reduce(
                    acc[:, col0:col0 + B], e3[:],
                    axis=mybir.AxisListType.X, op=mybir.AluOpType.add,
                )
            # output in pieces so the final DMA only covers the tail;
            # gpsimd queue is free once inputs are loaded
            npieces = 8
            q = -(-ncols // npieces)
            for pz in range(npieces):
                c0, c1 = pz * q, min((pz + 1) * q, ncols)
                if c0 < c1:
                    nc.gpsimd.dma_start(out_d[:, c0:c1], acc[:, c0:c1])
    nc.compile()
    return nc


def _assemble(res, meta):
    ntiles, Ng = meta["ntiles"], meta["Ng"]
    dens_sorted = np.zeros(ntiles * P, np.float32)
    tilemap, slot_cols = meta["tilemap"], meta["slot_cols"]
    for c in range(NCORES):
        o = res.results[c]["out"]
        for k in range(meta["nslots"]):
            t = int(tilemap[k, c])
            v = o[:, slot_cols[k]].sum(axis=1, dtype=np.float64)
            dens_sorted[t * P:(t + 1) * P] = v.astype(np.float32)
    dens = np.zeros(Ng, np.float32)
    dens[meta["order_padded"][:Ng]] = dens_sorted[:Ng]
    side = round(Ng ** (1 / 3))
    if side ** 3 == Ng:
        return dens.reshape(side, side, side)
    return dens


def _in_maps(Gc, Wc, meta):
    maps = []
    for c in range(NCORES):
        m = {}
        for i in range(meta["nchunks"]):
            m[f"g{i}"] = np.ascontiguousarray(Gc[c][i])
            m[f"w{i}"] = np.ascontiguousarray(Wc[c][i])
        maps.append(m)
    return maps


def kernel(grid_points, X, aw_table, bw_table, elements, C_expand):
    Gc, Wc, meta = _prepare(grid_points, X, aw_table, bw_table,
                            elements, C_expand)
    nc = _build_program(meta)
    res = run_bass_kernel_spmd(nc, _in_maps(Gc, Wc, meta),
                               list(range(NCORES)))
    return _assemble(res, meta)


# revision 18
# speedup vs baseline: 1.0937x; 1.0937x over previous
"""Trainium2 Bass kernel for the Gaussian density calculator.

density[g] = sum_a mask_a * sum_n aw[e_a,n] * exp(bw[e_a,n] * ||g - X_a||^2)

Strategy (self-contained; hardcoded for 8 NeuronCores):
 - Host: drop masked atoms (they contribute exactly 0), spatially sort the
   grid points into 128-point tiles, and for every tile build the list of
   (atom, gaussian) pairs whose contribution can exceed exp(-CUT) anywhere
   in the tile (|bw| * d_min^2 <= CUT, d_min = distance from atom to the
   tile's bounding box).  Dropped terms are < 1e-6 relative -- far below
   fp32 resolution of the result.
 - The exponent is affine in per-point features:
       arg = bw*|g'|^2 - 2bw*(g'.X') + bw*|X'|^2 + log(aw)
           = [ |g'|^2, g'x, g'y, g'z, 1 ] . W[:, pair]
   (coordinates recentred per tile, aw folded into the exponent as log(aw)).
   On device: K=5 matmul per tile -> exp on ScalarE -> pair-sum on VectorE.
 - fp32-accurate matmul on the bf16 PE datapath: both operands split into
   3 bf16 components; the 6 cross products with |error| >= 2^-27 stack
   along the contraction dim (K = 30 <= 32, one PE row group).
 - Tiles are dealt to the 8 cores by workload rank (SPMD: identical
   instruction stream, near-balanced data); similar-sized tiles batch into
   one PSUM bank so a single ACTIVATE + one 3D-AP TENSOR_REDUCE serve the
   whole batch.  All matmuls of a batch share one PE row group (HW
   requirement for bank sharing); groups rotate across batches.
 - Operands stream in column-chunks so compute overlaps the input DMA.
"""
import numpy as np
import ml_dtypes

import concourse.bacc as bacc
import concourse.tile as tile
from concourse import mybir
from concourse.bass_utils import run_bass_kernel_spmd

P = 128
NCORES = 8
EXCLUDED_ELEM = 5
CUT = 16.0
MM_MAX = 512            # cols per matmul (one PSUM bank, fp32)
ITEM_MAX = 1024         # pair cols per batch item (2 banks, singleton batch)
BATCH_MAX = 512         # pair cols per multi-item batch (one PSUM bank --
                        # a matmul output must never straddle a bank)
BATCH_SLOTS = 16        # max items per batch
NCHUNKS = 2             # input DMA column chunks (compute/DMA overlap)
NEG_BIG = -1e30
NGROUPS = 3             # usable PE row groups for K<=32: {0,32,64}
BF16 = ml_dtypes.bfloat16


def _split3(x):
    a0 = x.astype(BF16)
    r1 = x - a0.astype(np.float64)
    a1 = r1.astype(BF16)
    r2 = r1 - a1.astype(np.float64)
    a2 = r2.astype(BF16)
    return a0, a1, a2


def _g_band(g5, k15):
    g0, g1, g2 = _split3(g5)
    if k15:                       # g exactly bf16: products (00),(01),(02)
        return np.concatenate([g0, g0, g0], axis=0)
    return np.concatenate([g0, g1, g2, g0, g0, g1], axis=0)


def _w_band(w5, k15):
    w0, w1, w2 = _split3(w5)
    if k15:
        return np.concatenate([w0, w1, w2], axis=0)
    return np.concatenate([w0, w1, w0, w1, w2, w0], axis=0)


def _prepare(grid_points, X, aw_table, bw_table, elements, C_expand):
    gp = grid_points.astype(np.float64)
    Ng = gp.shape[0]

    mask = (elements != EXCLUDED_ELEM) & (C_expand == 1)
    Xa = X.astype(np.float64)[mask]
    el = elements[mask]
    aw = aw_table.astype(np.float64)[el]
    bw = bw_table.astype(np.float64)[el]
    with np.errstate(divide="ignore", invalid="ignore"):
        logaw = np.where(aw > 0, np.log(np.maximum(aw, 1e-300)), NEG_BIG)

    # ---- spatial sort into tiles of 128 points ----
    ntiles = -(-Ng // P)
    ntiles = -(-ntiles // NCORES) * NCORES
    cell = np.floor(gp / np.array([2.0, 2.0, 4.0]))
    order = np.lexsort((cell[:, 2], cell[:, 1], cell[:, 0]))
    npad = ntiles * P - Ng
    order_padded = np.concatenate([order, np.full(npad, order[-1], np.int64)])
    gp_s = gp[order_padded].reshape(ntiles, P, 3)

    lo = gp_s.min(axis=1)
    hi = gp_s.max(axis=1)
    center = (lo + hi) / 2

    # ---- per-tile (atom, gaussian) pair selection ----
    d = np.maximum(lo[:, None, :] - Xa[None], Xa[None] - hi[:, None, :])
    d2 = (np.maximum(d, 0.0) ** 2).sum(-1)
    incl = (-bw)[None] * d2[:, :, None] <= CUT            # [T,Na,6]
    cnt = incl.reshape(ntiles, -1).sum(1)

    # ---- deal tiles to cores by workload rank ----
    nslots = ntiles // NCORES
    rank = np.argsort(-cnt, kind="stable")
    tilemap = rank.reshape(nslots, NCORES)                # [k, c] -> tile id
    pad_k = np.maximum(cnt[tilemap].max(1), 2)
    pad_k = ((pad_k + 1) // 2) * 2

    # ---- split slots into items (<= ITEM_MAX pair cols each) ----
    items = []                                            # [slot, q0, size]
    for k in range(nslots):
        rem, q0 = int(pad_k[k]), 0
        while rem > 0:
            s = min(rem, ITEM_MAX)
            items.append([k, q0, s])
            q0 += s
            rem -= s

    # ---- pack items into batches (greedy; items arrive size-sorted) ----
    batches = []                                          # dict(n, items)
    cur, cur_n = [], 0
    for it in items:
        n = max(cur_n, it[2])
        if cur and (len(cur) >= BATCH_SLOTS or (len(cur) + 1) * n > BATCH_MAX):
            batches.append(dict(n=cur_n, items=cur))
            cur, cur_n = [], 0
            n = it[2]
        cur.append(it)
        cur_n = n
    if cur:
        batches.append(dict(n=cur_n, items=cur))

    # ---- assign chunks, acc columns, G/W cols; emit matmul descriptors ----
    # chunk 0 is small so compute starts early; the rest stream behind it
    total_g = sum(-(-b["n"] // MM_MAX) * len(b["items"]) for b in batches)
    fracs = [0.33, 0.33, 0.34]
    bounds = np.cumsum([f * total_g for f in fracs])

    chunks = []                                           # per chunk: counters
    acccol = 0
    gdone = 0
    prev_grp = -1
    slot_cols = [[] for _ in range(nslots)]
    for bidx, b in enumerate(batches):
        n = b["n"]
        nm = -(-n // MM_MAX)
        ci = min(int(np.searchsorted(bounds, gdone, side="right")),
                 len(fracs) - 1)
        gdone += nm * len(b["items"])
        while len(chunks) <= ci:
            chunks.append(dict(g=[0] * NGROUPS, w=[0] * NGROUPS))
        ch = chunks[ci]
        b["chunk"] = ci
        # least-loaded group in this chunk (balanced widths), but never the
        # previous batch's group (keeps consecutive batches on different PE
        # row groups so their matmuls overlap in the array)
        cand = sorted(range(NGROUPS), key=lambda g: (ch["w"][g], g))
        grp = cand[0] if cand[0] != prev_grp else cand[1]
        prev_grp = grp
        for item in b["items"]:
            k, q0, size = item
            item_mms = []
            c0 = 0
            while c0 < n:
                sz = min(MM_MAX, n - c0)
                item_mms.append(dict(grp=grp, gcol=ch["g"][grp],
                                     woff=ch["w"][grp], sz=sz, c0=c0))
                ch["g"][grp] += 1
                ch["w"][grp] += sz
                c0 += sz
            item.append(item_mms)
            item.append(acccol)
            slot_cols[k].append(acccol)
            acccol += 1
    ncols = acccol
    gw_i = [max(c["g"]) * P for c in chunks]              # per-chunk G width
    ww_i = [max(c["w"]) for c in chunks]                  # per-chunk W width
    nchunks = len(chunks)

    # ---- per-core operand arrays ----
    g5_all = np.empty((ntiles, 5, P))
    gprime = gp_s - center[:, None, :]
    g5_all[:, 0] = (gprime ** 2).sum(-1)
    g5_all[:, 1:4] = np.swapaxes(gprime, 1, 2)
    g5_all[:, 4] = 1.0
    # grid features exactly bf16-representable (regular lattice) -> the G
    # split degenerates and K drops from 30 to 15 (halves operand DMA)
    k15 = bool(np.all(g5_all == g5_all.astype(BF16).astype(np.float64)))
    krows = 15 if k15 else 30

    pair_an = [np.nonzero(incl[t]) for t in range(ntiles)]
    Gc = [[np.zeros((3 * krows, gw_i[i]), BF16) for i in range(nchunks)]
          for _ in range(NCORES)]
    Wc = [[np.zeros((3 * krows, ww_i[i]), BF16) for i in range(nchunks)]
          for _ in range(NCORES)]
    gband_cache = {}
    for b in batches:
        n, ci = b["n"], b["chunk"]
        for k, q0, size, item_mms, _col in b["items"]:
            for c in range(NCORES):
                t = int(tilemap[k, c])
                if t not in gband_cache:
                    gband_cache[t] = _g_band(g5_all[t], k15)
                aa, nn = pair_an[t]
                a_it = aa[q0:q0 + n]
                n_it = nn[q0:q0 + n]
                mi = a_it.shape[0]
                w5 = np.empty((5, n))
                w5[:, mi:] = np.array([0, 0, 0, 0, NEG_BIG])[:, None]
                if mi:
                    Xp = Xa[a_it] - center[t]
                    bwi = bw[a_it, n_it]
                    w5[0, :mi] = bwi
                    w5[1:4, :mi] = -2.0 * bwi * Xp.T
                    w5[4, :mi] = bwi * (Xp ** 2).sum(-1) + logaw[a_it, n_it]
                wb = _w_band(w5, k15)
                for mm in item_mms:
                    p0 = krows * mm["grp"]
                    Gc[c][ci][p0:p0 + krows,
                              mm["gcol"] * P:(mm["gcol"] + 1) * P] = \
                        gband_cache[t]
                    Wc[c][ci][p0:p0 + krows,
                              mm["woff"]:mm["woff"] + mm["sz"]] = \
                        wb[:, mm["c0"]:mm["c0"] + mm["sz"]]

    meta = dict(
        nslots=nslots, ncols=ncols, batches=batches, slot_cols=slot_cols,
        gw_i=gw_i, ww_i=ww_i, nchunks=nchunks, pad_k=pad_k, krows=krows,
        tilemap=tilemap, order_padded=order_padded, Ng=Ng, ntiles=ntiles,
    )
    return Gc, Wc, meta


def _build_program(meta):
    nc = bacc.Bacc("TRN2", target_bir_lowering=False, debug=False,
                   num_devices=NCORES)
    ncols = meta["ncols"]
    gw_i, ww_i = meta["gw_i"], meta["ww_i"]
    nchunks = meta["nchunks"]
    krows = meta["krows"]
    g_d = [nc.dram_tensor(f"g{i}", [3 * krows, gw_i[i]], mybir.dt.bfloat16,
                          kind="ExternalInput") for i in range(nchunks)]
    w_d = [nc.dram_tensor(f"w{i}", [3 * krows, ww_i[i]], mybir.dt.bfloat16,
                          kind="ExternalInput") for i in range(nchunks)]
    out_d = nc.dram_tensor("out", [P, ncols], mybir.dt.float32,
                           kind="ExternalOutput")

    with tile.TileContext(nc) as tc:
        with (
            tc.tile_pool(name="data", bufs=1) as data,
            tc.tile_pool(name="ps", bufs=4, space="PSUM") as ps,
            tc.tile_pool(name="work", bufs=4) as work,
        ):
            g_sb, w_sb = [], []
            for i in range(nchunks):
                gt = data.tile([P, gw_i[i]], mybir.dt.bfloat16, tag=f"g{i}")
                wt = data.tile([P, ww_i[i]], mybir.dt.bfloat16, tag=f"w{i}")
                # move only the 3 used 32-aligned bands, spread over the two
                # fast HWDGE queues (each serves issue order -> chunk 0
                # lands first); gpsimd's slow SWDGE only carries the output
                for g in range(NGROUPS):
                    qg = (nc.sync, nc.scalar)[g % 2]
                    qw = (nc.scalar, nc.sync)[g % 2]
                    qg.dma_start(gt[32 * g:32 * g + krows, :],
                                 g_d[i][krows * g:krows * (g + 1), :])
                    qw.dma_start(wt[32 * g:32 * g + krows, :],
                                 w_d[i][krows * g:krows * (g + 1), :])
                g_sb.append(gt)
                w_sb.append(wt)
            acc = data.tile([P, ncols], mybir.dt.float32)
            for b in meta["batches"]:
                n, bi, ci = b["n"], b["items"], b["chunk"]
                B = len(bi)
                ps3 = ps.tile([P, B, n], mybir.dt.float32, tag="arg")
                e3 = work.tile([P, B, n], mybir.dt.float16, tag="e")
                for bidx, (k, q0, size, item_mms, _col) in enumerate(bi):
                    for mm in item_mms:
                        p0 = 32 * mm["grp"]
                        nc.tensor.matmul(
                            ps3[:, bidx, mm["c0"]:mm["c0"] + mm["sz"]],
                            g_sb[ci][p0:p0 + krows,
                                     mm["gcol"] * P:(mm["gcol"] + 1) * P],
                            w_sb[ci][p0:p0 + krows,
                                     mm["woff"]:mm["woff"] + mm["sz"]],
                            start=True, stop=True,
                        )
                nc.scalar.activation(out=e3[:], in_=ps3[:],
                                     func=mybir.ActivationFunctionType.Exp)
                col0 = bi[0][4]
                nc.vector.tensor_reduce(
                    acc[:, col0:col0 + B], e3[:],
                    axis=mybir.AxisListType.X, op=mybir.AluOpType.add,
                )
            # output in pieces so the final DMA only covers the tail;
            # gpsimd queue is free once inputs are loaded
            npieces = 8
            q = -(-ncols // npieces)
            for pz in range(npieces):
                c0, c1 = pz * q, min((pz + 1) * q, ncols)
                if c0 < c1:
                    nc.gpsimd.dma_start(out_d[:, c0:c1], acc[:, c0:c1])
    nc.compile()
    return nc


def _assemble(res, meta):
    ntiles, Ng = meta["ntiles"], meta["Ng"]
    dens_sorted = np.zeros(ntiles * P, np.float32)
    tilemap, slot_cols = meta["tilemap"], meta["slot_cols"]
    for c in range(NCORES):
        o = res.results[c]["out"]
        for k in range(meta["nslots"]):
            t = int(tilemap[k, c])
            v = o[:, slot_cols[k]].sum(axis=1, dtype=np.float64)
            dens_sorted[t * P:(t + 1) * P] = v.astype(np.float32)
    dens = np.zeros(Ng, np.float32)
    dens[meta["order_padded"][:Ng]] = dens_sorted[:Ng]
    side = round(Ng ** (1 / 3))
    if side ** 3 == Ng:
        return dens.reshape(side, side, side)
    return dens


def _in_maps(Gc, Wc, meta):
    maps = []
    for c in range(NCORES):
        m = {}
        for i in range(meta["nchunks"]):
            m[f"g{i}"] = np.ascontiguousarray(Gc[c][i])
            m[f"w{i}"] = np.ascontiguousarray(Wc[c][i])
        maps.append(m)
    return maps


def kernel(grid_points, X, aw_table, bw_table, elements, C_expand):
    Gc, Wc, meta = _prepare(grid_points, X, aw_table, bw_table,
                            elements, C_expand)
    nc = _build_program(meta)
    res = run_bass_kernel_spmd(nc, _in_maps(Gc, Wc, meta),
                               list(range(NCORES)))
    return _assemble(res, meta)


# revision 19
# speedup vs baseline: 1.2504x; 1.1433x over previous
"""Trainium2 Bass kernel for the Gaussian density calculator.

density[g] = sum_a mask_a * sum_n aw[e_a,n] * exp(bw[e_a,n] * ||g - X_a||^2)

Strategy (self-contained; hardcoded for 8 NeuronCores):
 - Host: drop masked atoms (they contribute exactly 0), spatially sort the
   grid points into 128-point tiles, and for every tile build the list of
   (atom, gaussian) pairs whose contribution can exceed exp(-CUT) anywhere
   in the tile (|bw| * d_min^2 <= CUT, d_min = distance from atom to the
   tile's bounding box).  Dropped terms are < 1e-6 relative -- far below
   fp32 resolution of the result.
 - The exponent is affine in per-point features:
       arg = bw*|g'|^2 - 2bw*(g'.X') + bw*|X'|^2 + log(aw)
           = [ |g'|^2, g'x, g'y, g'z, 1 ] . W[:, pair]
   (coordinates recentred per tile, aw folded into the exponent as log(aw)).
   On device: K=5 matmul per tile -> exp on ScalarE -> pair-sum on VectorE.
 - fp32-accurate matmul on the bf16 PE datapath: both operands split into
   3 bf16 components; the 6 cross products with |error| >= 2^-27 stack
   along the contraction dim (K = 30 <= 32, one PE row group).
 - Tiles are dealt to the 8 cores by workload rank (SPMD: identical
   instruction stream, near-balanced data); similar-sized tiles batch into
   one PSUM bank so a single ACTIVATE + one 3D-AP TENSOR_REDUCE serve the
   whole batch.  All matmuls of a batch share one PE row group (HW
   requirement for bank sharing); groups rotate across batches.
 - Operands stream in column-chunks so compute overlaps the input DMA.
"""
import numpy as np
import ml_dtypes

import concourse.bacc as bacc
import concourse.tile as tile
from concourse import mybir
from concourse.bass_utils import run_bass_kernel_spmd

P = 128
NCORES = 8
EXCLUDED_ELEM = 5
CUT = 16.0
MM_MAX = 512            # cols per matmul (one PSUM bank, fp32)
ITEM_MAX = 1024         # pair cols per batch item (2 banks, singleton batch)
BATCH_MAX = 512         # pair cols per multi-item batch (one PSUM bank --
                        # a matmul output must never straddle a bank)
BATCH_SLOTS = 16        # max items per batch
NCHUNKS = 2             # input DMA column chunks (compute/DMA overlap)
NEG_BIG = -1e30
NGROUPS = 3             # usable PE row groups for K<=32: {0,32,64}
BF16 = ml_dtypes.bfloat16


def _split3(x):
    a0 = x.astype(BF16)
    r1 = x - a0.astype(np.float64)
    a1 = r1.astype(BF16)
    r2 = r1 - a1.astype(np.float64)
    a2 = r2.astype(BF16)
    return a0, a1, a2


def _g_band(g5, k15):
    g0, g1, g2 = _split3(g5)
    if k15:                       # g exactly bf16: products (00),(01),(02)
        return np.concatenate([g0, g0, g0], axis=0)
    return np.concatenate([g0, g1, g2, g0, g0, g1], axis=0)


def _w_band(w5, k15):
    w0, w1, w2 = _split3(w5)
    if k15:
        return np.concatenate([w0, w1, w2], axis=0)
    return np.concatenate([w0, w1, w0, w1, w2, w0], axis=0)


def _prepare(grid_points, X, aw_table, bw_table, elements, C_expand):
    gp = grid_points.astype(np.float64)
    Ng = gp.shape[0]

    mask = (elements != EXCLUDED_ELEM) & (C_expand == 1)
    Xa = X.astype(np.float64)[mask]
    el = elements[mask]
    aw = aw_table.astype(np.float64)[el]
    bw = bw_table.astype(np.float64)[el]
    with np.errstate(divide="ignore", invalid="ignore"):
        logaw = np.where(aw > 0, np.log(np.maximum(aw, 1e-300)), NEG_BIG)

    # ---- spatial sort into tiles of 128 points ----
    ntiles = -(-Ng // P)
    ntiles = -(-ntiles // NCORES) * NCORES
    cell = np.floor(gp / np.array([2.0, 2.0, 4.0]))
    order = np.lexsort((cell[:, 2], cell[:, 1], cell[:, 0]))
    npad = ntiles * P - Ng
    order_padded = np.concatenate([order, np.full(npad, order[-1], np.int64)])
    gp_s = gp[order_padded].reshape(ntiles, P, 3)

    lo = gp_s.min(axis=1)
    hi = gp_s.max(axis=1)
    center = (lo + hi) / 2

    # ---- per-tile (atom, gaussian) pair selection ----
    d = np.maximum(lo[:, None, :] - Xa[None], Xa[None] - hi[:, None, :])
    d2 = (np.maximum(d, 0.0) ** 2).sum(-1)
    incl = (-bw)[None] * d2[:, :, None] <= CUT            # [T,Na,6]
    cnt = incl.reshape(ntiles, -1).sum(1)

    # ---- deal tiles to cores by workload rank ----
    nslots = ntiles // NCORES
    rank = np.argsort(-cnt, kind="stable")
    tilemap = rank.reshape(nslots, NCORES)                # [k, c] -> tile id
    pad_k = np.maximum(cnt[tilemap].max(1), 2)
    pad_k = ((pad_k + 1) // 2) * 2

    # ---- split slots into items (<= ITEM_MAX pair cols each) ----
    items = []                                            # [slot, q0, size]
    for k in range(nslots):
        rem, q0 = int(pad_k[k]), 0
        while rem > 0:
            s = min(rem, ITEM_MAX)
            items.append([k, q0, s])
            q0 += s
            rem -= s

    # ---- pack items into batches (greedy; items arrive size-sorted) ----
    batches = []                                          # dict(n, items)
    cur, cur_n = [], 0
    for it in items:
        n = max(cur_n, it[2])
        if cur and (len(cur) >= BATCH_SLOTS or (len(cur) + 1) * n > BATCH_MAX):
            batches.append(dict(n=cur_n, items=cur))
            cur, cur_n = [], 0
            n = it[2]
        cur.append(it)
        cur_n = n
    if cur:
        batches.append(dict(n=cur_n, items=cur))

    # ---- assign chunks, acc columns, G/W cols; emit matmul descriptors ----
    # chunk 0 is small so compute starts early; the rest stream behind it
    total_g = sum(-(-b["n"] // MM_MAX) * len(b["items"]) for b in batches)
    fracs = [0.33, 0.33, 0.34]
    bounds = np.cumsum([f * total_g for f in fracs])

    # G pattern table: after per-tile recentring the lattice makes most
    # tiles share one identical feature block, so the stationary operand
    # is a tiny shared table instead of a per-slot stream
    g5_all = np.empty((ntiles, 5, P))
    gprime = gp_s - center[:, None, :]
    g5_all[:, 0] = (gprime ** 2).sum(-1)
    g5_all[:, 1:4] = np.swapaxes(gprime, 1, 2)
    g5_all[:, 4] = 1.0
    k15 = bool(np.all(g5_all == g5_all.astype(BF16).astype(np.float64)))
    krows = 15 if k15 else 30
    pat_of_tile = {}
    pat_ids = {}
    for t in range(ntiles):
        key = g5_all[t].tobytes()
        pat_of_tile[t] = pat_ids.setdefault(key, len(pat_ids))
    shared_col = {}                                       # pattern -> gcol
    gcol_next = 0

    chunks = []                                           # per chunk: counters
    acccol = 0
    gdone = 0
    prev_grp = -1
    slot_cols = [[] for _ in range(nslots)]
    for bidx, b in enumerate(batches):
        n = b["n"]
        nm = -(-n // MM_MAX)
        ci = min(int(np.searchsorted(bounds, gdone, side="right")),
                 len(fracs) - 1)
        gdone += nm * len(b["items"])
        while len(chunks) <= ci:
            chunks.append(dict(g=[0] * NGROUPS, w=[0] * NGROUPS))
        ch = chunks[ci]
        b["chunk"] = ci
        # least-loaded group in this chunk (balanced widths), but never the
        # previous batch's group (keeps consecutive batches on different PE
        # row groups so their matmuls overlap in the array)
        cand = sorted(range(NGROUPS), key=lambda g: (ch["w"][g], g))
        grp = cand[0] if cand[0] != prev_grp else cand[1]
        prev_grp = grp
        for item in b["items"]:
            k, q0, size = item
            pats = {pat_of_tile[int(tilemap[k, c])] for c in range(NCORES)}
            if len(pats) == 1:
                p = pats.pop()
                if p not in shared_col:
                    shared_col[p] = gcol_next
                    gcol_next += 1
                gcol = shared_col[p]
            else:                    # per-core private pattern column
                gcol = gcol_next
                gcol_next += 1
            item_mms = []
            c0 = 0
            while c0 < n:
                sz = min(MM_MAX, n - c0)
                item_mms.append(dict(grp=grp, gcol=gcol,
                                     woff=ch["w"][grp], sz=sz, c0=c0))
                ch["w"][grp] += sz
                c0 += sz
            item.append(item_mms)
            item.append(acccol)
            slot_cols[k].append(acccol)
            acccol += 1
    ncols = acccol
    GPW = gcol_next * P                                   # pattern-table width
    ww_i = [max(c["w"]) for c in chunks]                  # per-chunk W width
    nchunks = len(chunks)

    # ---- per-core operand arrays ----

    pair_an = [np.nonzero(incl[t]) for t in range(ntiles)]
    Gc = [np.zeros((3 * krows, GPW), BF16) for _ in range(NCORES)]
    Wc = [[np.zeros((3 * krows, ww_i[i]), BF16) for i in range(nchunks)]
          for _ in range(NCORES)]
    gband_cache = {}
    for b in batches:
        n, ci = b["n"], b["chunk"]
        for k, q0, size, item_mms, _col in b["items"]:
            for c in range(NCORES):
                t = int(tilemap[k, c])
                if t not in gband_cache:
                    gband_cache[t] = _g_band(g5_all[t], k15)
                aa, nn = pair_an[t]
                a_it = aa[q0:q0 + n]
                n_it = nn[q0:q0 + n]
                mi = a_it.shape[0]
                w5 = np.empty((5, n))
                w5[:, mi:] = np.array([0, 0, 0, 0, NEG_BIG])[:, None]
                if mi:
                    Xp = Xa[a_it] - center[t]
                    bwi = bw[a_it, n_it]
                    w5[0, :mi] = bwi
                    w5[1:4, :mi] = -2.0 * bwi * Xp.T
                    w5[4, :mi] = bwi * (Xp ** 2).sum(-1) + logaw[a_it, n_it]
                wb = _w_band(w5, k15)
                for mm in item_mms:
                    p0 = krows * mm["grp"]
                    for band in range(NGROUPS):
                        Gc[c][krows * band:krows * (band + 1),
                              mm["gcol"] * P:(mm["gcol"] + 1) * P] = \
                            gband_cache[t]
                    Wc[c][ci][p0:p0 + krows,
                              mm["woff"]:mm["woff"] + mm["sz"]] = \
                        wb[:, mm["c0"]:mm["c0"] + mm["sz"]]

    meta = dict(
        nslots=nslots, ncols=ncols, batches=batches, slot_cols=slot_cols,
        GPW=GPW, ww_i=ww_i, nchunks=nchunks, pad_k=pad_k, krows=krows,
        tilemap=tilemap, order_padded=order_padded, Ng=Ng, ntiles=ntiles,
    )
    return Gc, Wc, meta


def _build_program(meta):
    nc = bacc.Bacc("TRN2", target_bir_lowering=False, debug=False,
                   num_devices=NCORES)
    ncols = meta["ncols"]
    GPW, ww_i = meta["GPW"], meta["ww_i"]
    nchunks = meta["nchunks"]
    krows = meta["krows"]
    g_d = nc.dram_tensor("gp", [3 * krows, GPW], mybir.dt.bfloat16,
                         kind="ExternalInput")
    w_d = [nc.dram_tensor(f"w{i}", [3 * krows, ww_i[i]], mybir.dt.bfloat16,
                          kind="ExternalInput") for i in range(nchunks)]
    out_d = nc.dram_tensor("out", [P, ncols], mybir.dt.float32,
                           kind="ExternalOutput")

    with tile.TileContext(nc) as tc:
        with (
            tc.tile_pool(name="data", bufs=1) as data,
            tc.tile_pool(name="ps", bufs=4, space="PSUM") as ps,
            tc.tile_pool(name="work", bufs=4) as work,
        ):
            g_sb = data.tile([P, GPW], mybir.dt.bfloat16)
            for g in range(NGROUPS):
                nc.sync.dma_start(g_sb[32 * g:32 * g + krows, :],
                                  g_d[krows * g:krows * (g + 1), :])
            w_sb = []
            for i in range(nchunks):
                wt = data.tile([P, ww_i[i]], mybir.dt.bfloat16, tag=f"w{i}")
                # 3 used 32-aligned bands, spread over the two fast HWDGE
                # queues (each serves issue order -> chunk 0 lands first)
                for g in range(NGROUPS):
                    qw = (nc.scalar, nc.sync)[g % 2]
                    qw.dma_start(wt[32 * g:32 * g + krows, :],
                                 w_d[i][krows * g:krows * (g + 1), :])
                w_sb.append(wt)
            acc = data.tile([P, ncols], mybir.dt.float32)
            for b in meta["batches"]:
                n, bi, ci = b["n"], b["items"], b["chunk"]
                B = len(bi)
                ps3 = ps.tile([P, B, n], mybir.dt.float32, tag="arg")
                e3 = work.tile([P, B, n], mybir.dt.float16, tag="e")
                for bidx, (k, q0, size, item_mms, _col) in enumerate(bi):
                    for mm in item_mms:
                        p0 = 32 * mm["grp"]
                        nc.tensor.matmul(
                            ps3[:, bidx, mm["c0"]:mm["c0"] + mm["sz"]],
                            g_sb[p0:p0 + krows,
                                 mm["gcol"] * P:(mm["gcol"] + 1) * P],
                            w_sb[ci][p0:p0 + krows,
                                     mm["woff"]:mm["woff"] + mm["sz"]],
                            start=True, stop=True,
                        )
                nc.scalar.activation(out=e3[:], in_=ps3[:],
                                     func=mybir.ActivationFunctionType.Exp)
                col0 = bi[0][4]
                nc.vector.tensor_reduce(
                    acc[:, col0:col0 + B], e3[:],
                    axis=mybir.AxisListType.X, op=mybir.AluOpType.add,
                )
            # output in pieces so the final DMA only covers the tail;
            # gpsimd queue is free once inputs are loaded
            npieces = 8
            q = -(-ncols // npieces)
            for pz in range(npieces):
                c0, c1 = pz * q, min((pz + 1) * q, ncols)
                if c0 < c1:
                    nc.gpsimd.dma_start(out_d[:, c0:c1], acc[:, c0:c1])
    nc.compile()
    return nc


def _assemble(res, meta):
    ntiles, Ng = meta["ntiles"], meta["Ng"]
    dens_sorted = np.zeros(ntiles * P, np.float32)
    tilemap, slot_cols = meta["tilemap"], meta["slot_cols"]
    for c in range(NCORES):
        o = res.results[c]["out"]
        for k in range(meta["nslots"]):
            t = int(tilemap[k, c])
            v = o[:, slot_cols[k]].sum(axis=1, dtype=np.float64)
            dens_sorted[t * P:(t + 1) * P] = v.astype(np.float32)
    dens = np.zeros(Ng, np.float32)
    dens[meta["order_padded"][:Ng]] = dens_sorted[:Ng]
    side = round(Ng ** (1 / 3))
    if side ** 3 == Ng:
        return dens.reshape(side, side, side)
    return dens


def _in_maps(Gc, Wc, meta):
    maps = []
    for c in range(NCORES):
        m = {}
        m["gp"] = np.ascontiguousarray(Gc[c])
        for i in range(meta["nchunks"]):
            m[f"w{i}"] = np.ascontiguousarray(Wc[c][i])
        maps.append(m)
    return maps


def kernel(grid_points, X, aw_table, bw_table, elements, C_expand):
    Gc, Wc, meta = _prepare(grid_points, X, aw_table, bw_table,
                            elements, C_expand)
    nc = _build_program(meta)
    res = run_bass_kernel_spmd(nc, _in_maps(Gc, Wc, meta),
                               list(range(NCORES)))
    return _assemble(res, meta)
